# revision 5
# baseline (speedup 1.0000x reference)
"""Trainium2 Bass kernel for AttentionAggregationNN (ragged single-query MHA pooling).

Algebraic reduction: with one shared query vector, softmax-attention pooling per
group collapses to, per instance i and head h:
    e[i,h]   = exp(s_h . x_i)          (score logits; softmax shift-invariance
                                        lets the constant term drop)
    val[i,h] = t_h . x_i
    out[g]   = sum_h (sum_{i in g} e*val) / (sum_{i in g} e) + CONST
where s_h = Wk_h^T q_h / sqrt(D), t_h = Wv_h^T (w_lin @ w_out)_h, all folded on
the host in float64.

Device work per core (data-parallel over groups, pre-sorted):
  mm1: SP[128,16] = XT_chunk.T @ W16          (scores | vals)
  ACT: e = exp(scores);  DVE: ev = e * vals
  one-hot M[i,j] = (rel_gid[i] == j) via tensor_scalar is_equal
  mm2: acc[16, win] += [e|ev].T @ M           (segment sums into PSUM)
  epilogue: transpose, divide, reduce, +CONST, DMA out.
"""
import os

if os.environ.get("AXON_H4_ENABLED") == "1" or os.environ.get("AXON_TERMINAL_JOB_NAME"):
    # run_bass_via_pjrt needs the axon jax platform; make sure it is loadable
    # even if the caller pinned JAX_PLATFORMS=cpu (harness-safe, no-op if jax
    # was already imported with axon available).
    plats = os.environ.get("JAX_PLATFORMS", "")
    if "axon" not in plats:
        os.environ["JAX_PLATFORMS"] = "axon,cpu"

import numpy as np

# ---------------------------------------------------------------- problem dims
N, G, E, H, D = 131072, 2048, 256, 8, 32
NCORES = 8
GC = G // NCORES        # 256 groups per core
P = 128                 # partition dim / subtile rows
WIN = 64                # one-hot window width (host asserts span fits)
BATCH = 4               # subtiles per ACT/DVE batch
MACRO = 4096            # rows per DMA macro-tile

USE_BF16 = True         # compute dtype for X / W16 / one-hot / mm2 operands

_CACHE: dict = {}


# ---------------------------------------------------------------- host algebra
def _fold_params(query, w_in, b_in, w_out, b_out, w_lin, b_lin):
    q64 = query.reshape(E).astype(np.float64)
    w64, b64 = w_in.astype(np.float64), b_in.astype(np.float64)
    wq, wk, wv = w64[:E], w64[E:2 * E], w64[2 * E:]
    bq, bk, bv = b64[:E], b64[E:2 * E], b64[2 * E:]
    q = wq @ q64 + bq
    qh = q.reshape(H, D)
    S = np.einsum("hde,hd->he", wk.reshape(H, D, E), qh) / np.sqrt(D)
    u = (w_lin.astype(np.float64) @ w_out.astype(np.float64)).reshape(E)
    uh = u.reshape(H, D)
    T = np.einsum("hde,hd->he", wv.reshape(H, D, E), uh)
    const = float(np.einsum("hd,hd->", uh, bv.reshape(H, D))
                  + w_lin.astype(np.float64).reshape(E) @ b_out.astype(np.float64)
                  + b_lin.astype(np.float64)[0])
    W16 = np.concatenate([S.T, T.T], axis=1)    # [E, 16]
    return W16, const


def _shard_prep(tree_preds, group_ids, np_dtype):
    sizes = np.bincount(group_ids, minlength=G)
    offsets = np.concatenate([[0], np.cumsum(sizes)]).astype(np.int64)
    sorter = np.argsort(group_ids, kind="stable")
    Xs = np.ascontiguousarray(tree_preds[sorter])
    gs = group_ids[sorter].astype(np.int64)

    row_start = offsets[np.arange(NCORES) * GC]
    row_end = offsets[(np.arange(NCORES) + 1) * GC]
    rows = row_end - row_start
    rows_cap = int(np.ceil(rows.max() / (P * BATCH)) * (P * BATCH))
    nsub = rows_cap // P

    grel = np.full((NCORES, rows_cap), -1, np.int64)
    for c in range(NCORES):
        grel[c, :rows[c]] = gs[row_start[c]:row_end[c]] - c * GC
    gsub = grel.reshape(NCORES, nsub, P)
    lo = np.where(gsub >= 0, gsub, G).min(axis=(0, 2))
    hi = np.where(gsub >= 0, gsub, -1).max(axis=(0, 2))
    span = np.where(hi >= 0, hi - np.minimum(lo, hi) + 1, 1)
    assert span.max() <= WIN, f"one-hot window span {span.max()} exceeds WIN={WIN}"
    woff = np.minimum(np.where(lo < G, lo, 0), GC - WIN).astype(np.int64)
    assert ((hi < woff + WIN) | (hi < 0)).all()

    rel = np.where(gsub >= 0, gsub - woff[None, :, None], WIN).astype(np.float32)
    RELT = np.ascontiguousarray(rel.transpose(0, 2, 1)).astype(np.float32)  # [NC, P, nsub]

    XT = np.zeros((NCORES, 2, P, rows_cap), np_dtype)
    for c in range(NCORES):
        blk = Xs[row_start[c]:row_end[c]].T.astype(np_dtype)   # [256, rows_c]
        XT[c, 0, :, :rows[c]] = blk[:P]
        XT[c, 1, :, :rows[c]] = blk[P:]
    return XT, RELT, woff, rows_cap, nsub


# ---------------------------------------------------------------- bass program
def _build_program(rows_cap, nsub, woff, const):
    import concourse.bass as bass
    import concourse.tile as tile
    from concourse import bacc, mybir
    from concourse.masks import make_identity

    DT = mybir.dt.bfloat16 if USE_BF16 else mybir.dt.float32
    F32 = mybir.dt.float32
    Exp = mybir.ActivationFunctionType.Exp
    Alu = mybir.AluOpType

    nc = bacc.Bacc(None, target_bir_lowering=False)
    xt = nc.dram_tensor("xt", [2, P, rows_cap], DT, kind="ExternalInput")
    relt = nc.dram_tensor("relt", [P, nsub], F32, kind="ExternalInput")
    wmat = nc.dram_tensor("wmat", [2, P, 16], DT, kind="ExternalInput")
    jiota = nc.dram_tensor("jiota", [P, WIN], DT, kind="ExternalInput")
    out = nc.dram_tensor("out", [GC], F32, kind="ExternalOutput")

    with tile.TileContext(nc) as tc:
        with (
            tc.tile_pool(name="const", bufs=1) as constp,
            tc.tile_pool(name="xtp", bufs=2) as xtp,
            tc.tile_pool(name="work", bufs=3) as workp,
            tc.tile_pool(name="ep", bufs=1) as epsb,
            tc.tile_pool(name="mm1", bufs=4, space="PSUM") as mm1p,
            tc.tile_pool(name="acc", bufs=1, space="PSUM") as accp,
            tc.tile_pool(name="tps", bufs=2, space="PSUM") as tpsp,
        ):
            # ---- constants
            w_t = constp.tile([P, 32], DT)
            nc.sync.dma_start(w_t[:, 0:16], wmat[0])
            nc.sync.dma_start(w_t[:, 16:32], wmat[1])
            j_t = constp.tile([P, WIN], DT)
            nc.sync.dma_start(j_t[:], jiota[:])
            relt_t = constp.tile([P, nsub], F32)
            nc.sync.dma_start(relt_t[:], relt[:])
            zbias = constp.tile([P, 1], F32)
            nc.gpsimd.memset(zbias[:], 0.0)
            zw = constp.tile([P, 16], DT)
            nc.gpsimd.memset(zw[:], 0.0)
            ident = constp.tile([16, 16], F32)
            make_identity(nc, ident[:])

            # ---- segment-sum accumulator, zeroed via zero-weight matmuls
            acc = accp.tile([16, GC], F32)
            for w0 in range(0, GC, WIN):
                nc.tensor.matmul(acc[:, w0:w0 + WIN], lhsT=zw[:], rhs=j_t[:],
                                 start=True, stop=False, skip_group_check=True)

            # ---- main loop
            s = 0
            m0 = 0
            while m0 < rows_cap:
                msz = min(MACRO, rows_cap - m0)
                x0 = xtp.tile([P, msz], DT, tag="x0")
                x1 = xtp.tile([P, msz], DT, tag="x1")
                nc.sync.dma_start(x0[:], xt[0, :, m0:m0 + msz])
                nc.sync.dma_start(x1[:], xt[1, :, m0:m0 + msz])
                for b0 in range(0, msz, P * BATCH):
                    spp = mm1p.tile([P, 16 * BATCH], F32)
                    m_t = workp.tile([P, BATCH * WIN], DT, tag="m")
                    sp2 = workp.tile([P, 16 * BATCH], DT, tag="sp2")
                    for j in range(BATCH):
                        col = b0 + j * P
                        nc.tensor.matmul(spp[:, j * 16:j * 16 + 16],
                                         lhsT=x0[:, col:col + P], rhs=w_t[:, 0:16],
                                         start=True, stop=False)
                        nc.tensor.matmul(spp[:, j * 16:j * 16 + 16],
                                         lhsT=x1[:, col:col + P], rhs=w_t[:, 16:32],
                                         start=False, stop=True)
                        nc.vector.tensor_scalar(
                            m_t[:, j * WIN:(j + 1) * WIN], j_t[:],
                            relt_t[:, s + j:s + j + 1], None, op0=Alu.is_equal)
                    spv = spp[:].rearrange("p (b c) -> p b c", b=BATCH)
                    sp2v = sp2[:].rearrange("p (b c) -> p b c", b=BATCH)
                    nc.scalar.activation(sp2v[:, :, 0:8], spv[:, :, 0:8], Exp,
                                         bias=zbias[:])
                    nc.vector.tensor_tensor(sp2v[:, :, 8:16], sp2v[:, :, 0:8],
                                            spv[:, :, 8:16], op=Alu.mult)
                    for j in range(BATCH):
                        nc.tensor.matmul(
                            acc[:, woff[s + j]:woff[s + j] + WIN],
                            lhsT=sp2[:, j * 16:j * 16 + 16],
                            rhs=m_t[:, j * WIN:(j + 1) * WIN],
                            start=False, stop=(s + j == nsub - 1),
                            skip_group_check=True)
                    s += BATCH
                m0 += msz

            # ---- epilogue: acc[16, GC] -> out[GC]
            cc = epsb.tile([16, GC], F32)
            nc.vector.tensor_copy(cc[:], acc[:])
            dd = epsb.tile([P, 32], F32)
            for c in range(GC // P):
                tp = tpsp.tile([P, 16], F32)
                nc.tensor.transpose(tp[:], cc[:, c * P:(c + 1) * P], ident[:])
                nc.vector.tensor_copy(dd[:, c * 16:(c + 1) * 16], tp[:])
            ddv = dd[:].rearrange("p (c k) -> p c k", c=2)
            rec = epsb.tile([P, 16], F32)
            recv = rec[:].rearrange("p (c k) -> p c k", c=2)
            nc.vector.reciprocal(recv[:, :, :], ddv[:, :, 0:8])
            rr = epsb.tile([P, 16], F32)
            rrv = rr[:].rearrange("p (c k) -> p c k", c=2)
            nc.vector.tensor_tensor(rrv[:, :, :], recv[:, :, :], ddv[:, :, 8:16],
                                    op=Alu.mult)
            oo = epsb.tile([P, 2], F32)
            nc.vector.tensor_reduce(oo[:], rrv[:, :, :], axis=mybir.AxisListType.X,
                                    op=Alu.add)
            oo2 = epsb.tile([P, 2], F32)
            nc.vector.tensor_scalar_add(oo2[:], oo[:], float(const))
            for c in range(GC // P):
                nc.sync.dma_start(out[c * P:(c + 1) * P, None], oo2[:, c:c + 1])
    nc.compile()
    return nc


# ---------------------------------------------------------------- entry point
def _invoke(tree_preds, group_ids, query, w_in, b_in, w_out, b_out, w_lin, b_lin,
            trace=False, **spmd_kwargs):
    import ml_dtypes
    np_dt = ml_dtypes.bfloat16 if USE_BF16 else np.float32

    tree_preds = np.asarray(tree_preds, dtype=np.float32)
    group_ids = np.asarray(group_ids, dtype=np.int32)

    W16, const = _fold_params(np.asarray(query), np.asarray(w_in), np.asarray(b_in),
                              np.asarray(w_out), np.asarray(b_out),
                              np.asarray(w_lin), np.asarray(b_lin))
    XT, RELT, woff, rows_cap, nsub = _shard_prep(tree_preds, group_ids, np_dt)

    key = (rows_cap, nsub, tuple(woff.tolist()), float(const))
    if _CACHE.get("key") != key:
        _CACHE["nc"] = _build_program(rows_cap, nsub, woff, const)
        _CACHE["key"] = key
    nc = _CACHE["nc"]

    wmat = np.ascontiguousarray(W16.astype(np_dt).reshape(2, P, 16))
    jio = np.broadcast_to(np.arange(WIN, dtype=np.float32), (P, WIN)).astype(np_dt)
    jio = np.ascontiguousarray(jio)

    in_maps = []
    for c in range(NCORES):
        in_maps.append({
            "xt": XT[c],
            "relt": RELT[c],
            "wmat": wmat,
            "jiota": jio,
        })

    from concourse.bass_utils import run_bass_kernel_spmd
    res = run_bass_kernel_spmd(nc, in_maps, core_ids=list(range(NCORES)),
                               trace=trace, **spmd_kwargs)

    out = np.empty((G, 1), np.float32)
    for c in range(NCORES):
        out[c * GC:(c + 1) * GC, 0] = res.results[c]["out"]
    return out, res


def kernel(tree_preds, group_ids, query, w_in, b_in, w_out, b_out, w_lin, b_lin):
    out, _ = _invoke(tree_preds, group_ids, query, w_in, b_in,
                     w_out, b_out, w_lin, b_lin)
    return out
